# revision 5
# baseline (speedup 1.0000x reference)
"""Trainium2 Bass kernel for nn_ContMlpPerFeature.

Computes, per feature f (32 of them):
    h = relu(r_[:, f, :] @ W1[f] + b1[f])     # [B, 128]
    y = relu(h @ W2[f] + b2[f])               # [B, 1]
    out[:, f, 0] = X[:, f]; out[:, f, 1] = y

Sharding: pure data-parallel over batch (B=16384) across 8 cores.

Design notes (v2):
  - X never touches the device: out[...,0] is X verbatim, host interleaves
    it during unshard.  The device computes y only.
  - b1 is folded into the L1 matmul via a ones-row: rpt tiles are
    [65, 2048] bf16 per feature with row 64 = 1.0 (packed on host), and
    w1aug is [65, F, H] with row 64 = b1.  Contraction depth is free in
    the PE cost model (cost = output columns), so the bias costs nothing
    and the PSUM->SBUF drains become pure relu.  That decouples drain
    spans from feature boundaries.
  - PSUM: 1 bank yT accumulator + 7 banks of h slots in a 3-slot
    rotation (1536, 1024, 1024 cols).  Three slots stagger fill/drain so
    each slot's serial fill+drain latency hides behind the other two.
  - Drains (the kernel bottleneck: only ACT/DVE can read PSUM, at
    1 elem/cycle/partition for fp32) are assigned to engines greedily by
    cumulative busy time (ACT 0.833 ns/elem + 185 ns; DVE 1.042 ns/elem
    + 125 ns), not strict alternation: balanced finish, ~1.9 elem/ns
    combined.
  - Startup taper: first spans are 512 cols so both engines engage as
    soon as the first rpt piece lands.  First rpt DMA is split
    (512 cols, then 1536) to cut first-fill latency.
  - L2 uses h as the stationary operand (ldweights is free): per
    128-col batch chunk, a single-column matmul accumulates
    y^T[128b, 1] into the whole-batch yT bank (b2 pre-loaded by a K=1
    ones x b2row matmul).  L2 is emitted a few spans behind its drain so
    the PE queue head never blocks.
  - y is drained in 4 pieces (f 0:16, 16:24, 24:30, 30:32) so the final
    relu+DMA tail after the last drain is minimal.
  - PE p-state warm-up matmuls (into the yT bank, later overwritten by
    the b2 init) burn the 3 us ramp clock during the initial DMAs.
"""

from collections import deque

import ml_dtypes
import numpy as np

import concourse.bass as bass
import concourse.tile as tile
from concourse import bacc, mybir
from concourse.bass_utils import run_bass_kernel_spmd

F32 = mybir.dt.float32
BF16 = mybir.dt.bfloat16

N_CORES = 8
B_FULL, F, D = 16384, 32, 64
H = 2 * D  # 128
K = D + 1  # contraction with the ones/bias row

# y piece boundaries in feature space (last piece tiny for a short tail)
Y_PIECES = [(0, 16), (16, 24), (24, 30), (30, 32)]

# per-element engine costs (ns) + per-instruction overheads, for greedy
# engine balancing (mirrors the TimelineSim cost model)
DVE_ELEM, DVE_INIT = 1.0417, 125.0
ACT_ELEM, ACT_INIT = 0.8333, 185.0


def _span_schedule(total_cols: int):
    """(slot_size, span_size) pairs.  Slots rotate A(1536), B#0(1024),
    B#1(1024) — 7 PSUM banks total; spans may use a slot partially
    (startup taper).  All sizes are multiples of 512."""
    spans = [(1536, 512), (1024, 512), (1024, 512)]  # taper cycle
    left = total_cols - 1536
    while left > 3584:
        spans += [(1536, 1536), (1024, 1024), (1024, 1024)]
        left -= 3584
    # remainder (multiple of 512, <= 3584) over one last slot cycle
    for slot in (1536, 1024, 1024):
        take = min(left, slot)
        if take:
            spans.append((slot, take))
            left -= take
    assert left == 0 and sum(s for _, s in spans) == total_cols
    return spans


def _build_nc(Bl: int) -> bass.Bass:
    n_feat_cols = 2048  # Bl per feature
    assert Bl == n_feat_cols
    nbt = Bl // 128  # 16 batch tiles per feature
    total_cols = F * Bl  # 65536 global columns

    nc = bacc.Bacc()

    rT = nc.dram_tensor("rT", [F, K, Bl], BF16, kind="ExternalInput")
    w1a = nc.dram_tensor("w1a", [K, F, H], BF16, kind="ExternalInput")
    w2T = nc.dram_tensor("w2T", [H, F], BF16, kind="ExternalInput")
    # cb[0, 0:nbt*F] = b2 replicated per batch-tile; cb[0, nbt*F:+H] = ones
    cb = nc.dram_tensor("cb", [1, nbt * F + H], BF16, kind="ExternalInput")
    y_out = [
        nc.dram_tensor(f"y{i}", [128, nbt, fb - fa], F32, kind="ExternalOutput")
        for i, (fa, fb) in enumerate(Y_PIECES)
    ]

    with tile.TileContext(nc) as tc:
        with (
            tc.tile_pool(name="singles", bufs=1) as singles,
            tc.tile_pool(name="rpt", bufs=1) as p_rpt,
            tc.tile_pool(name="h", bufs=1) as p_h,
            tc.tile_pool(name="y", bufs=1) as p_y,
            tc.tile_pool(name="hps", bufs=1, space="PSUM") as p_hps,
            tc.tile_pool(name="yps", bufs=1, space="PSUM") as p_yps,
        ):
            # ---- static SBUF tiles -----------------------------------
            w1a_t = singles.tile([K, F, H], BF16)
            w2T_t = singles.tile([128, F], BF16)
            cb_t = singles.tile([1, nbt * F + H], BF16)
            rpt_t = [
                p_rpt.tile([K, Bl], BF16, bufs=1, tag=f"rpt{f}", name=f"rpt{f}")
                for f in range(F)
            ]

            # ---- input DMAs (order = just-in-time for the pipeline) --
            nc.sync.dma_start(out=w1a_t[:, 0:2, :], in_=w1a[:, 0:2, :])
            nc.sync.dma_start(out=rpt_t[0][:, 0:512], in_=rT[0, :, 0:512])
            nc.sync.dma_start(out=w2T_t, in_=w2T[:])
            nc.sync.dma_start(out=cb_t, in_=cb[:])
            nc.sync.dma_start(out=rpt_t[0][:, 512:Bl], in_=rT[0, :, 512:Bl])
            nc.sync.dma_start(out=rpt_t[1], in_=rT[1])
            nc.sync.dma_start(out=w1a_t[:, 2:, :], in_=w1a[:, 2:, :])
            for f in range(2, F):
                nc.sync.dma_start(out=rpt_t[f], in_=rT[f])

            b2row = cb_t[:, 0 : nbt * F]
            ones = cb_t[:, nbt * F : nbt * F + H]

            # ---- PSUM: yT accumulator bank + 7-bank h slot rotation --
            yT = p_yps.tile([128, nbt * F], F32, tag="yT")
            yT_v = yT.rearrange("p (g f) -> p g f", f=F)

            # PE p-state warm-up into the yT bank (overwritten by the b2
            # init matmul before any L2 accumulation).
            warm_sb = singles.tile([128, 512], BF16)
            nc.gpsimd.memset(warm_sb, 0.0)
            for _ in range(7):
                nc.tensor.matmul(
                    yT, lhsT=warm_sb[:, 0:128], rhs=warm_sb,
                    start=True, stop=True, skip_group_check=True,
                )

            # preload the ACT Relu table (hidden behind the input DMAs)
            act_warm = singles.tile([128, 2], F32)
            nc.gpsimd.memset(act_warm[:, 0:1], 0.0)
            nc.scalar.activation(
                act_warm[:, 1:2], act_warm[:, 0:1],
                mybir.ActivationFunctionType.Relu,
            )

            # ---- main pipeline ---------------------------------------
            spans = _span_schedule(total_cols)
            eng_load = {"dve": 0.0, "act": 800.0}  # ACT also runs y relus

            def pick_engine(size):
                t_d = eng_load["dve"] + size * DVE_ELEM + DVE_INIT
                t_a = eng_load["act"] + size * ACT_ELEM + ACT_INIT
                if t_a <= t_d:
                    eng_load["act"] = t_a
                    return True  # use ACT
                eng_load["dve"] = t_d
                return False

            def emit_piece(pi):
                fa, fb = Y_PIECES[pi]
                y_sb = p_y.tile(
                    [128, nbt * (fb - fa)], F32, bufs=1, tag=f"yp{pi}",
                    name=f"y_sb{pi}",
                )
                nc.scalar.activation(
                    y_sb, yT_v[:, :, fa:fb],
                    mybir.ActivationFunctionType.Relu,
                )
                eng_load["act"] += nbt * (fb - fa) * ACT_ELEM + ACT_INIT
                nc.sync.dma_start(out=y_out[pi][:], in_=y_sb)

            yt_inited = False
            next_piece = 0
            pending = deque()  # (global_col0, size, h_sb)

            def emit_l2(flush_to_col):
                nonlocal yt_inited, next_piece
                while pending and (
                    flush_to_col is None or pending[0][0] < flush_to_col
                ):
                    if not yt_inited:
                        nc.tensor.matmul(
                            yT, lhsT=ones, rhs=b2row, start=True, stop=False,
                            skip_group_check=True,
                        )
                        yt_inited = True
                    g0, size, h_sb = pending.popleft()
                    for j in range(size // 128):
                        g = g0 + j * 128
                        f = g // n_feat_cols
                        bt = (g % n_feat_cols) // 128
                        col = bt * F + f
                        nc.tensor.matmul(
                            yT[:, col : col + 1],
                            lhsT=h_sb[:, 128 * j : 128 * (j + 1)],
                            rhs=w2T_t[:, f : f + 1],
                            start=False,
                            stop=True,
                            skip_group_check=True,
                        )
                    # fire y pieces as soon as their feature range is done
                    while (
                        next_piece < len(Y_PIECES)
                        and g0 + size >= Y_PIECES[next_piece][1] * n_feat_cols
                    ):
                        emit_piece(next_piece)
                        next_piece += 1

            g = 0  # global column cursor
            for si, (slot, size) in enumerate(spans):
                # h slot tile (PSUM) + its SBUF mirror; spans may use the
                # slot partially during the startup taper
                h_ps_full = p_hps.tile(
                    [128, slot], F32, bufs=(1 if slot == 1536 else 2),
                    tag=f"ps{slot}", name=f"ps{slot}",
                )
                h_sb_full = p_h.tile(
                    [128, slot], BF16, bufs=(3 if slot == 1536 else 4),
                    tag=f"hs{slot}", name=f"hs{slot}",
                )
                h_ps = h_ps_full[:, 0:size]
                h_sb = h_sb_full[:, 0:size]
                # fills: 512-col matmuls (bank-contained), K=65 with bias row
                for off in range(0, size, 512):
                    gg = g + off
                    f = gg // n_feat_cols
                    foff = gg % n_feat_cols
                    nc.tensor.matmul(
                        h_ps[:, off : off + 512],
                        lhsT=w1a_t[:, f, :],
                        rhs=rpt_t[f][:, foff : foff + 512],
                        start=True,
                        stop=True,
                        tile_position=(0, 0),
                    )
                # drain: pure relu (bias already in PSUM), balanced engine
                if pick_engine(size):
                    nc.scalar.activation(
                        h_sb, h_ps, mybir.ActivationFunctionType.Relu
                    )
                else:
                    nc.vector.tensor_scalar(
                        out=h_sb,
                        in0=h_ps,
                        scalar1=0.0,
                        scalar2=None,
                        op0=mybir.AluOpType.max,
                    )
                pending.append((g, size, h_sb))
                g += size
                # L2 lags ~2 spans so PE's queue head never waits on a drain
                emit_l2(g - 3072)
            emit_l2(None)

    nc.compile()
    return nc


_NC_CACHE: dict[int, bass.Bass] = {}


def _get_nc(Bl: int) -> bass.Bass:
    if Bl not in _NC_CACHE:
        _NC_CACHE[Bl] = _build_nc(Bl)
    return _NC_CACHE[Bl]


def _host_pack_weights(W1, b1, W2, b2, Bl):
    """Shared (replicated) device inputs, pre-packed for large-descriptor
    DMAs and the kernel's on-chip layouts."""
    nbt = Bl // 128
    # w1a: [K=65, F, H]; rows 0:64 = W1[f], row 64 = b1[f]
    w1a = np.zeros((K, F, H), dtype=ml_dtypes.bfloat16)
    w1a[:D] = (
        np.asarray(W1, dtype=np.float32)
        .transpose(1, 0, 2)
        .astype(ml_dtypes.bfloat16)
    )
    w1a[D] = np.asarray(b1, dtype=np.float32).astype(ml_dtypes.bfloat16)
    w2T = np.ascontiguousarray(
        np.asarray(W2, dtype=np.float32).reshape(F, H).T.astype(ml_dtypes.bfloat16)
    )  # [H, F]
    b2f = np.asarray(b2, dtype=np.float32).reshape(F)
    cb = np.zeros((1, nbt * F + H), dtype=ml_dtypes.bfloat16)
    cb[0, : nbt * F] = np.tile(b2f, nbt).astype(ml_dtypes.bfloat16)
    cb[0, nbt * F :] = np.float32(1.0)
    return np.ascontiguousarray(w1a), w2T, cb


def _run(X, r_, W1, b1, W2, b2, trace=False, **spmd_kwargs):
    X = np.ascontiguousarray(np.asarray(X, dtype=np.float32))
    r_ = np.asarray(r_, dtype=np.float32)

    Btot = X.shape[0]
    assert Btot % N_CORES == 0
    Bl = Btot // N_CORES
    nbt = Bl // 128
    w1a, w2T, cb = _host_pack_weights(W1, b1, W2, b2, Bl)
    nc = _get_nc(Bl)

    in_maps = []
    for i in range(N_CORES):
        sl = slice(i * Bl, (i + 1) * Bl)
        # transpose + cast + ones-row append is part of host-side sharding:
        # [Bl,F,D] -> [F,K,Bl] with row 64 = 1.0
        rTc = np.empty((F, K, Bl), dtype=ml_dtypes.bfloat16)
        rTc[:, :D, :] = r_[sl].transpose(1, 2, 0).astype(ml_dtypes.bfloat16)
        rTc[:, D, :] = np.float32(1.0)
        in_maps.append({"rT": rTc, "w1a": w1a, "w2T": w2T, "cb": cb})
    res = run_bass_kernel_spmd(
        nc, in_maps, core_ids=list(range(N_CORES)), trace=trace, **spmd_kwargs
    )
    # unshard: out[...,0] = X (host-side), out[...,1] = y from device
    out = np.empty((Btot, F, 2), dtype=np.float32)
    out[:, :, 0] = X
    for i in range(N_CORES):
        r = res.results[i]
        # [128, nbt, nf] pieces; b = bt*128 + p
        y = np.concatenate(
            [r[f"y{pi}"] for pi in range(len(Y_PIECES))], axis=-1
        )
        out[i * Bl : (i + 1) * Bl, :, 1] = y.transpose(1, 0, 2).reshape(Bl, F)
    return out, res


def kernel(X, r_, W1, b1, W2, b2):
    out, _ = _run(X, r_, W1, b1, W2, b2)
    return out


# revision 10
# speedup vs baseline: 1.0984x; 1.0984x over previous
"""Trainium2 Bass kernel for nn_ContMlpPerFeature.

Computes, per feature f (32 of them):
    h = relu(r_[:, f, :] @ W1[f] + b1[f])     # [B, 128]
    y = relu(h @ W2[f] + b2[f])               # [B, 1]
    out[:, f, 0] = X[:, f]; out[:, f, 1] = y

Sharding: pure data-parallel over batch (B=16384) across 8 cores.

Design notes (v2):
  - X never touches the device: out[...,0] is X verbatim, host interleaves
    it during unshard.  The device computes y only.
  - b1 is folded into the L1 matmul via a ones-row: rpt tiles are
    [65, 2048] bf16 per feature with row 64 = 1.0 (packed on host), and
    w1aug is [65, F, H] with row 64 = b1.  Contraction depth is free in
    the PE cost model (cost = output columns), so the bias costs nothing
    and the PSUM->SBUF drains become pure relu.  That decouples drain
    spans from feature boundaries.
  - PSUM: 1 bank yT accumulator + 7 banks of h slots in a 3-slot
    rotation (1536, 1024, 1024 cols).  Three slots stagger fill/drain so
    each slot's serial fill+drain latency hides behind the other two.
  - Drains (the kernel bottleneck: only ACT/DVE can read PSUM, at
    1 elem/cycle/partition for fp32) are assigned to engines greedily by
    cumulative busy time (ACT 0.833 ns/elem + 185 ns; DVE 1.042 ns/elem
    + 125 ns), not strict alternation: balanced finish, ~1.9 elem/ns
    combined.
  - Startup taper: first spans are 512 cols so both engines engage as
    soon as the first rpt piece lands.  First rpt DMA is split
    (512 cols, then 1536) to cut first-fill latency.
  - L2 uses h as the stationary operand (ldweights is free): per
    128-col batch chunk, a single-column matmul accumulates
    y^T[128b, 1] into the whole-batch yT bank (b2 pre-loaded by a K=1
    ones x b2row matmul).  L2 is emitted a few spans behind its drain so
    the PE queue head never blocks.
  - y is drained in 4 pieces (f 0:16, 16:24, 24:30, 30:32) so the final
    relu+DMA tail after the last drain is minimal.
  - PE p-state warm-up matmuls (into the yT bank, later overwritten by
    the b2 init) burn the 3 us ramp clock during the initial DMAs.
"""

from collections import deque

import ml_dtypes
import numpy as np

import concourse.bass as bass
import concourse.tile as tile
from concourse import bacc, mybir
from concourse.bass_utils import run_bass_kernel_spmd

F32 = mybir.dt.float32
BF16 = mybir.dt.bfloat16

N_CORES = 8
B_FULL, F, D = 16384, 32, 64
H = 2 * D  # 128
K = D + 1  # contraction with the ones/bias row

# y piece boundaries in feature space (last piece tiny for a short tail)
Y_PIECES = [(0, 16), (16, 24), (24, 30), (30, 32)]

# per-element engine costs (ns) + per-instruction overheads, for greedy
# engine balancing (mirrors the TimelineSim cost model)
DVE_ELEM, DVE_INIT = 1.0417, 125.0
ACT_ELEM, ACT_INIT = 0.8333, 185.0


def _span_schedule(total_cols: int):
    """Span sizes over three uniform 1024-col PSUM slots (uniform slots
    stagger fill/drain cleanly on the in-order PE).  512-col spans at both
    ends shorten the pipeline ramp and the final-drain tail."""
    n_full = (total_cols - 2048) // 1024
    spans = [512, 512] + [1024] * n_full + [512, 512]
    assert sum(spans) == total_cols
    return spans


def _build_nc(Bl: int) -> bass.Bass:
    n_feat_cols = 2048  # Bl per feature
    assert Bl == n_feat_cols
    nbt = Bl // 128  # 16 batch tiles per feature
    total_cols = F * Bl  # 65536 global columns

    nc = bacc.Bacc()

    rT = nc.dram_tensor("rT", [F, K, Bl], BF16, kind="ExternalInput")
    w1a = nc.dram_tensor("w1a", [K, F, H], BF16, kind="ExternalInput")
    w2T = nc.dram_tensor("w2T", [H, F], BF16, kind="ExternalInput")
    # cb[0, 0:nbt*F] = b2 replicated per batch-tile; cb[0, nbt*F:+H] = ones
    cb = nc.dram_tensor("cb", [1, nbt * F + H], BF16, kind="ExternalInput")
    y_out = [
        nc.dram_tensor(f"y{i}", [128, nbt, fb - fa], F32, kind="ExternalOutput")
        for i, (fa, fb) in enumerate(Y_PIECES)
    ]

    with tile.TileContext(nc) as tc:
        with (
            tc.tile_pool(name="singles", bufs=1) as singles,
            tc.tile_pool(name="rpt", bufs=1) as p_rpt,
            tc.tile_pool(name="h", bufs=1) as p_h,
            tc.tile_pool(name="y", bufs=1) as p_y,
            tc.tile_pool(name="hps", bufs=1, space="PSUM") as p_hps,
            tc.tile_pool(name="yps", bufs=1, space="PSUM") as p_yps,
        ):
            # ---- static SBUF tiles -----------------------------------
            w1a_t = singles.tile([K, F, H], BF16)
            w2T_t = singles.tile([128, F], BF16)
            cb_t = singles.tile([1, nbt * F + H], BF16)
            rpt_t = [
                p_rpt.tile([K, Bl], BF16, bufs=1, tag=f"rpt{f}", name=f"rpt{f}")
                for f in range(F)
            ]

            # ---- input DMAs (order = just-in-time for the pipeline) --
            nc.sync.dma_start(out=w1a_t[:, 0:2, :], in_=w1a[:, 0:2, :])
            nc.sync.dma_start(out=rpt_t[0][:, 0:512], in_=rT[0, :, 0:512])
            nc.sync.dma_start(out=w2T_t, in_=w2T[:])
            nc.sync.dma_start(out=cb_t, in_=cb[:])
            nc.sync.dma_start(out=rpt_t[0][:, 512:Bl], in_=rT[0, :, 512:Bl])
            nc.sync.dma_start(out=rpt_t[1], in_=rT[1])
            nc.sync.dma_start(out=w1a_t[:, 2:, :], in_=w1a[:, 2:, :])
            for f in range(2, F):
                nc.sync.dma_start(out=rpt_t[f], in_=rT[f])

            b2row = cb_t[:, 0 : nbt * F]
            ones = cb_t[:, nbt * F : nbt * F + H]

            # ---- PSUM: yT accumulator bank + 7-bank h slot rotation --
            yT = p_yps.tile([128, nbt * F], F32, tag="yT")
            yT_v = yT.rearrange("p (g f) -> p g f", f=F)

            # PE p-state warm-up into the yT bank (overwritten by the b2
            # init matmul before any L2 accumulation).
            warm_sb = singles.tile([128, 512], BF16)
            nc.gpsimd.memset(warm_sb, 0.0)
            for _ in range(7):
                nc.tensor.matmul(
                    yT, lhsT=warm_sb[:, 0:128], rhs=warm_sb,
                    start=True, stop=True, skip_group_check=True,
                )

            # preload the ACT Relu table (hidden behind the input DMAs)
            act_warm = singles.tile([128, 2], F32)
            nc.gpsimd.memset(act_warm[:, 0:1], 0.0)
            nc.scalar.activation(
                act_warm[:, 1:2], act_warm[:, 0:1],
                mybir.ActivationFunctionType.Relu,
            )

            # ---- main pipeline ---------------------------------------
            spans = _span_schedule(total_cols)
            eng_load = {"dve": 0.0, "act": 0.0}

            def pick_engine(size):
                t_d = eng_load["dve"] + size * DVE_ELEM + DVE_INIT
                t_a = eng_load["act"] + size * ACT_ELEM + ACT_INIT
                if t_a <= t_d:
                    eng_load["act"] = t_a
                    return True  # use ACT
                eng_load["dve"] = t_d
                return False

            def emit_piece(pi):
                fa, fb = Y_PIECES[pi]
                y_sb = p_y.tile(
                    [128, nbt * (fb - fa)], F32, bufs=1, tag=f"yp{pi}",
                    name=f"y_sb{pi}",
                )
                if pick_engine(nbt * (fb - fa)):
                    nc.scalar.activation(
                        y_sb, yT_v[:, :, fa:fb],
                        mybir.ActivationFunctionType.Relu,
                    )
                else:
                    nc.vector.tensor_scalar(
                        out=y_sb,
                        in0=yT_v[:, :, fa:fb],
                        scalar1=0.0,
                        scalar2=None,
                        op0=mybir.AluOpType.max,
                    )
                nc.sync.dma_start(out=y_out[pi][:], in_=y_sb)

            yt_inited = False
            next_piece = 0
            pending = deque()  # (global_col0, size, h_sb)

            def emit_l2(flush_to_col):
                nonlocal yt_inited, next_piece
                while pending and (
                    flush_to_col is None or pending[0][0] < flush_to_col
                ):
                    if not yt_inited:
                        nc.tensor.matmul(
                            yT, lhsT=ones, rhs=b2row, start=True, stop=False,
                            skip_group_check=True,
                        )
                        yt_inited = True
                    g0, size, h_sb = pending.popleft()
                    for j in range(size // 128):
                        g = g0 + j * 128
                        f = g // n_feat_cols
                        bt = (g % n_feat_cols) // 128
                        col = bt * F + f
                        nc.tensor.matmul(
                            yT[:, col : col + 1],
                            lhsT=h_sb[:, 128 * j : 128 * (j + 1)],
                            rhs=w2T_t[:, f : f + 1],
                            start=False,
                            stop=True,
                            skip_group_check=True,
                        )
                    # fire y pieces as soon as their feature range is done
                    while (
                        next_piece < len(Y_PIECES)
                        and g0 + size >= Y_PIECES[next_piece][1] * n_feat_cols
                    ):
                        emit_piece(next_piece)
                        next_piece += 1

            g = 0  # global column cursor
            for si, size in enumerate(spans):
                # h slot tile (PSUM) + its SBUF mirror; 512-col spans use
                # the slot partially (taper)
                h_ps_full = p_hps.tile(
                    [128, 1024], F32, bufs=3, tag="ps", name="ps",
                )
                h_sb_full = p_h.tile(
                    [128, 1024], BF16, bufs=12, tag="hs", name="hs",
                )
                h_ps = h_ps_full[:, 0:size]
                h_sb = h_sb_full[:, 0:size]
                # fills: 512-col matmuls (bank-contained), K=65 with bias row
                for off in range(0, size, 512):
                    gg = g + off
                    f = gg // n_feat_cols
                    foff = gg % n_feat_cols
                    nc.tensor.matmul(
                        h_ps[:, off : off + 512],
                        lhsT=w1a_t[:, f, :],
                        rhs=rpt_t[f][:, foff : foff + 512],
                        start=True,
                        stop=True,
                        tile_position=(0, 0),
                    )
                # drain: pure relu (bias already in PSUM), balanced engine
                if pick_engine(size):
                    nc.scalar.activation(
                        h_sb, h_ps, mybir.ActivationFunctionType.Relu
                    )
                else:
                    nc.vector.tensor_scalar(
                        out=h_sb,
                        in0=h_ps,
                        scalar1=0.0,
                        scalar2=None,
                        op0=mybir.AluOpType.max,
                    )
                pending.append((g, size, h_sb))
                g += size
                # L2 lags far behind the drains (h_sb is buffered 12 deep)
                # so the PE queue head never waits on an unfinished drain
                emit_l2(g - 8192)
            emit_l2(None)

    nc.compile()
    return nc


_NC_CACHE: dict[int, bass.Bass] = {}


def _get_nc(Bl: int) -> bass.Bass:
    if Bl not in _NC_CACHE:
        _NC_CACHE[Bl] = _build_nc(Bl)
    return _NC_CACHE[Bl]


def _host_pack_weights(W1, b1, W2, b2, Bl):
    """Shared (replicated) device inputs, pre-packed for large-descriptor
    DMAs and the kernel's on-chip layouts."""
    nbt = Bl // 128
    # w1a: [K=65, F, H]; rows 0:64 = W1[f], row 64 = b1[f]
    w1a = np.zeros((K, F, H), dtype=ml_dtypes.bfloat16)
    w1a[:D] = (
        np.asarray(W1, dtype=np.float32)
        .transpose(1, 0, 2)
        .astype(ml_dtypes.bfloat16)
    )
    w1a[D] = np.asarray(b1, dtype=np.float32).astype(ml_dtypes.bfloat16)
    w2T = np.ascontiguousarray(
        np.asarray(W2, dtype=np.float32).reshape(F, H).T.astype(ml_dtypes.bfloat16)
    )  # [H, F]
    b2f = np.asarray(b2, dtype=np.float32).reshape(F)
    cb = np.zeros((1, nbt * F + H), dtype=ml_dtypes.bfloat16)
    cb[0, : nbt * F] = np.tile(b2f, nbt).astype(ml_dtypes.bfloat16)
    cb[0, nbt * F :] = np.float32(1.0)
    return np.ascontiguousarray(w1a), w2T, cb


def _run(X, r_, W1, b1, W2, b2, trace=False, **spmd_kwargs):
    X = np.ascontiguousarray(np.asarray(X, dtype=np.float32))
    r_ = np.asarray(r_, dtype=np.float32)

    Btot = X.shape[0]
    assert Btot % N_CORES == 0
    Bl = Btot // N_CORES
    nbt = Bl // 128
    w1a, w2T, cb = _host_pack_weights(W1, b1, W2, b2, Bl)
    nc = _get_nc(Bl)

    in_maps = []
    for i in range(N_CORES):
        sl = slice(i * Bl, (i + 1) * Bl)
        # transpose + cast + ones-row append is part of host-side sharding:
        # [Bl,F,D] -> [F,K,Bl] with row 64 = 1.0
        rTc = np.empty((F, K, Bl), dtype=ml_dtypes.bfloat16)
        rTc[:, :D, :] = r_[sl].transpose(1, 2, 0).astype(ml_dtypes.bfloat16)
        rTc[:, D, :] = np.float32(1.0)
        in_maps.append({"rT": rTc, "w1a": w1a, "w2T": w2T, "cb": cb})
    res = run_bass_kernel_spmd(
        nc, in_maps, core_ids=list(range(N_CORES)), trace=trace, **spmd_kwargs
    )
    # unshard: out[...,0] = X (host-side), out[...,1] = y from device
    out = np.empty((Btot, F, 2), dtype=np.float32)
    out[:, :, 0] = X
    for i in range(N_CORES):
        r = res.results[i]
        # [128, nbt, nf] pieces; b = bt*128 + p
        y = np.concatenate(
            [r[f"y{pi}"] for pi in range(len(Y_PIECES))], axis=-1
        )
        out[i * Bl : (i + 1) * Bl, :, 1] = y.transpose(1, 0, 2).reshape(Bl, F)
    return out, res


def kernel(X, r_, W1, b1, W2, b2):
    out, _ = _run(X, r_, W1, b1, W2, b2)
    return out
